# revision 5
# baseline (speedup 1.0000x reference)
"""Trainium2 Bass kernel for the LELoss problem (raw Bass, 8-core SPMD).

loss = mean_b ||x_b - dec_b||^2
     + 1.1 * mean_b ||enc_b - (lat @ rsrA.T)_b||^2
     + 0.1 * mean((rsrA.T @ rsrA - I)^2)

(The knn/cdist/topk in the original module is dead code - its result is never
used - so the returned loss reduces to the three terms above.)

Per-core algebra (batch shard of R=1024 rows):
  sum||enc - lat@A.T||^2 = sum(enc^2) - 2*sum(M .* A) + sum(L .* G0)
      with M = enc.T @ lat [E,I], L = lat.T @ lat [I,I], G0 = A.T @ A [I,I]
  sum((G0 - I)^2) = sum(G0^2) - 2*sum(A^2) + I_dim
All partial sums land in columns of a [128,16] SBUF accumulator S; a single
ones-vector matmul collapses partitions to a [1,16] row per core. The host
sums the 8 rows and applies the weights.

Engine split: sync = DMA; vector = (x-dec) subtract + tiny fused
mul-accumulate reductions; scalar = Square-with-accumulate; PE = matmuls.
"""

import numpy as np

try:
    import concourse.bass as bass
except ImportError:  # pragma: no cover - grading env fallback
    import sys

    sys.path.insert(0, "/opt/trn_rl_repo")
    import concourse.bass as bass

from concourse import mybir
from concourse.bass_utils import run_bass_kernel_spmd

N_CORES = 8
B, D, E, I = 8192, 1024, 128, 20
R = B // N_CORES  # rows per core = 1024
P = 128  # SBUF partitions
RT = R // P  # row tiles per core = 8
S_COLS = 16
F32 = mybir.dt.float32

# test.py can flip these; the grading harness leaves defaults.
TRACE = False
LAST_RESULT = None

_NC = None


def _build_nc():
    nc = bass.Bass()
    x = nc.dram_tensor("x", [R, D], F32, kind="ExternalInput")
    dec = nc.dram_tensor("dec", [R, D], F32, kind="ExternalInput")
    enc = nc.dram_tensor("enc", [R, E], F32, kind="ExternalInput")
    lat = nc.dram_tensor("lat", [R, I], F32, kind="ExternalInput")
    rsra = nc.dram_tensor("rsra", [E, I], F32, kind="ExternalInput")
    out = nc.dram_tensor("out", [1, S_COLS], F32, kind="ExternalOutput")

    Square = mybir.ActivationFunctionType.Square
    mult = mybir.AluOpType.mult
    bypass = mybir.AluOpType.bypass

    enc_r = enc[:, :].rearrange("(n p) e -> p n e", p=P)  # [128, RT, E]
    lat_r = lat[:, :].rearrange("(n p) i -> p n i", p=P)  # [128, RT, I]

    import contextlib

    ctx = contextlib.ExitStack()
    with ctx:
        xb = [
            ctx.enter_context(nc.sbuf_tensor(f"xb{t}", [P, D], F32)) for t in range(RT)
        ]
        db = [
            ctx.enter_context(nc.sbuf_tensor(f"db{t}", [P, D], F32)) for t in range(RT)
        ]
        enc_sb = ctx.enter_context(nc.sbuf_tensor([P, RT * E], F32))
        lat_sb = ctx.enter_context(nc.sbuf_tensor([P, RT * I], F32))
        rsra_sb = ctx.enter_context(nc.sbuf_tensor([E, I], F32))
        S = ctx.enter_context(nc.sbuf_tensor([P, S_COLS], F32))
        ones = ctx.enter_context(nc.sbuf_tensor([P, 1], F32))
        G_sb = ctx.enter_context(nc.sbuf_tensor([I, I], F32))
        scr_m = ctx.enter_context(nc.sbuf_tensor([E, I], F32))
        scr_i = ctx.enter_context(nc.sbuf_tensor([I, I], F32))
        scr_a = ctx.enter_context(nc.sbuf_tensor([E, I], F32))
        scr_e = ctx.enter_context(nc.sbuf_tensor([P, RT * E], F32))
        out_sb = ctx.enter_context(nc.sbuf_tensor([1, S_COLS], F32))

        psum_M = ctx.enter_context(nc.psum_tensor([E, I], F32))
        psum_L = ctx.enter_context(nc.psum_tensor([I, I], F32))
        psum_G = ctx.enter_context(nc.psum_tensor([I, I], F32))
        psum_F = ctx.enter_context(nc.psum_tensor([1, S_COLS], F32))

        s_x = [ctx.enter_context(nc.semaphore(f"s_x{t}")) for t in range(RT)]
        s_small = ctx.enter_context(nc.semaphore("s_small"))
        s_init = ctx.enter_context(nc.semaphore("s_init"))
        s_sub = ctx.enter_context(nc.semaphore("s_sub"))
        s_sq = ctx.enter_context(nc.semaphore("s_sq"))
        s_pe = ctx.enter_context(nc.semaphore("s_pe"))
        s_vfin = ctx.enter_context(nc.semaphore("s_vfin"))
        s_fin = ctx.enter_context(nc.semaphore("s_fin"))
        s_out = ctx.enter_context(nc.semaphore("s_out"))

        block = ctx.enter_context(nc.Block())

        @block.sync
        def _(sync):
            # small tensors first: PE matmuls + enc^2/rA^2 run in the stream's shadow
            sync.dma_start(out=rsra_sb[:, :], in_=rsra[:, :]).then_inc(s_small, 16)
            sync.dma_start(
                out=enc_sb[:, :].rearrange("p (n e) -> p n e", n=RT), in_=enc_r
            ).then_inc(s_small, 16)
            sync.dma_start(
                out=lat_sb[:, :].rearrange("p (n i) -> p n i", n=RT), in_=lat_r
            ).then_inc(s_small, 16)
            # dominant stream: x/dec row tiles
            for t in range(RT):
                sync.dma_start(
                    out=xb[t][:, :], in_=x[t * P : (t + 1) * P, :]
                ).then_inc(s_x[t], 16)
                sync.dma_start(
                    out=db[t][:, :], in_=dec[t * P : (t + 1) * P, :]
                ).then_inc(s_x[t], 16)
            # result row out
            sync.wait_ge(s_fin, 1)
            sync.dma_start(out=out[:, :], in_=out_sb[:, :]).then_inc(s_out, 16)
            sync.wait_ge(s_out, 16)

        @block.vector
        def _(vector):
            nc.vector.memset(ones[:, :], 1.0)
            nc.vector.memset(S[:, :], 0.0).then_inc(s_init, 1)
            # tiny fused reductions over the PCA/proj matmul results
            vector.wait_ge(s_pe, 1)
            nc.vector.tensor_copy(G_sb[:, :], psum_G[:, :])
            # S[:,9] = rowsum(M .* rsrA)
            nc.vector.scalar_tensor_tensor(
                out=scr_m[:, :], in0=psum_M[:, :], scalar=1.0, in1=rsra_sb[:, :],
                op0=bypass, op1=mult, accum_out=S[:E, 9:10],
            )
            # S[0:I,10] = rowsum(L .* G0)
            nc.vector.scalar_tensor_tensor(
                out=scr_i[:, :], in0=psum_L[:, :], scalar=1.0, in1=G_sb[:, :],
                op0=bypass, op1=mult, accum_out=S[:I, 10:11],
            )
            # S[0:I,11] = rowsum(G0 .* G0)
            nc.vector.scalar_tensor_tensor(
                out=scr_i[:, :], in0=G_sb[:, :], scalar=1.0, in1=G_sb[:, :],
                op0=bypass, op1=mult, accum_out=S[:I, 11:12],
            ).then_inc(s_vfin, 1)
            # the big stream: d = x - dec, in place
            for t in range(RT):
                vector.wait_ge(s_x[t], 32)
                nc.vector.tensor_sub(xb[t][:, :], xb[t][:, :], db[t][:, :]).then_inc(
                    s_sub, 1
                )
            # ship the final row
            vector.wait_ge(s_pe, 2)
            nc.vector.tensor_copy(out_sb[:, :], psum_F[:, :]).then_inc(s_fin, 1)

        @block.scalar
        def _(scalar):
            # small squares while x/dec still stream in
            scalar.wait_ge(s_small, 48)
            scalar.wait_ge(s_init, 1)
            nc.scalar.activation(
                out=scr_a[:, :], in_=rsra_sb[:, :], func=Square,
                accum_out=S[:E, 12:13],
            ).then_inc(s_sq, 1)
            nc.scalar.activation(
                out=scr_e[:, :], in_=enc_sb[:, :], func=Square,
                accum_out=S[:, 8:9],
            ).then_inc(s_sq, 1)
            for t in range(RT):
                scalar.wait_ge(s_sub, t + 1)
                nc.scalar.activation(
                    out=db[t][:, :], in_=xb[t][:, :], func=Square,
                    accum_out=S[:, t : t + 1],
                ).then_inc(s_sq, 1)

        @block.tensor
        def _(tensor):
            tensor.wait_ge(s_small, 48)
            for t in range(RT):
                nc.tensor.matmul(
                    psum_M[:, :],
                    lhsT=enc_sb[:, t * E : (t + 1) * E],
                    rhs=lat_sb[:, t * I : (t + 1) * I],
                    start=(t == 0),
                    stop=(t == RT - 1),
                )
            for t in range(RT):
                nc.tensor.matmul(
                    psum_L[:, :],
                    lhsT=lat_sb[:, t * I : (t + 1) * I],
                    rhs=lat_sb[:, t * I : (t + 1) * I],
                    start=(t == 0),
                    stop=(t == RT - 1),
                )
            nc.tensor.matmul(
                psum_G[:, :], lhsT=rsra_sb[:, :], rhs=rsra_sb[:, :],
                start=True, stop=True,
            ).then_inc(s_pe, 1)
            # final partition-collapse: [1,16] = ones.T @ S
            tensor.wait_ge(s_sq, RT + 2)
            tensor.wait_ge(s_vfin, 1)
            nc.tensor.matmul(
                psum_F[:, :], lhsT=ones[:, :], rhs=S[:, :], start=True, stop=True
            ).then_inc(s_pe, 1)

    return nc


def kernel(x, encoded, latent, decoded, rsrA):
    global _NC, LAST_RESULT
    if _NC is None:
        _NC = _build_nc()

    x = np.ascontiguousarray(x, dtype=np.float32)
    decoded = np.ascontiguousarray(decoded, dtype=np.float32)
    encoded = np.ascontiguousarray(encoded, dtype=np.float32)
    latent = np.ascontiguousarray(latent, dtype=np.float32)
    rsrA = np.ascontiguousarray(rsrA, dtype=np.float32)

    in_maps = []
    for c in range(N_CORES):
        sl = slice(c * R, (c + 1) * R)
        in_maps.append(
            {
                "x": x[sl],
                "dec": decoded[sl],
                "enc": encoded[sl],
                "lat": latent[sl],
                "rsra": rsrA,
            }
        )

    res = run_bass_kernel_spmd(_NC, in_maps, core_ids=list(range(N_CORES)), trace=TRACE)
    LAST_RESULT = res

    o = np.stack([r["out"][0] for r in res.results]).astype(np.float64)  # [8,16]
    s_recon = o[:, 0:8].sum()
    s_enc2 = o[:, 8].sum()
    s_cross = o[:, 9].sum()
    s_zsq = o[:, 10].sum()
    g2 = o[0, 11]
    ra2 = o[0, 12]

    pca_sq = s_enc2 - 2.0 * s_cross + s_zsq
    proj_sq = g2 - 2.0 * ra2 + float(I)
    loss = s_recon / B + 1.1 * pca_sq / B + 0.1 * proj_sq / (I * I)
    return np.asarray(loss, dtype=np.float32)


# revision 6
# speedup vs baseline: 1.0900x; 1.0900x over previous
"""Trainium2 Bass kernel for the LELoss problem (raw Bass, 8-core SPMD).

loss = mean_b ||x_b - dec_b||^2
     + 1.1 * mean_b ||enc_b - (lat @ rsrA.T)_b||^2
     + 0.1 * mean((rsrA.T @ rsrA - I)^2)

(The knn/cdist/topk in the original module is dead code - its result is never
used - so the returned loss reduces to the three terms above.)

Per-core algebra (batch shard of R=1024 rows):
  sum||enc - lat@A.T||^2 = sum(enc^2) - 2*sum(M .* A) + sum(L .* G0)
      with M = enc.T @ lat [E,I], L = lat.T @ lat [I,I], G0 = A.T @ A [I,I]
  sum((G0 - I)^2) = sum(G0^2) - 2*sum(A^2) + I_dim
All partial sums land in columns of a [128,16] SBUF accumulator S which is
DMA'd out per core; the host collapses partitions/cores and applies weights.

DMA strategy: the two HWDGE queues (SP and ACT engines) each stream half the
~8.5MB/core - x tiles + half of enc on SP, dec tiles + the rest of the small
tensors on ACT - so HBM (~358 GB/s/core) is the only limiter. GpSimd is
avoided (SWDGE descriptor generation is too slow for these access patterns).

Engine split: SP = x DMAs + result DMA; ACT(scalar) = dec/small DMAs +
Square-with-accumulate; DVE(vector) = (x-dec) subtract + tiny fused
mul-accumulate reductions; PE = matmuls.
"""

import contextlib

import numpy as np

try:
    import concourse.bass as bass
except ImportError:  # pragma: no cover - grading env fallback
    import sys

    sys.path.insert(0, "/opt/trn_rl_repo")
    import concourse.bass as bass

from concourse import mybir
from concourse.bass_utils import run_bass_kernel_spmd

N_CORES = 8
B, D, E, I = 8192, 1024, 128, 20
R = B // N_CORES  # rows per core = 1024
P = 128  # SBUF partitions
RT = R // P  # row tiles per core = 8
S_COLS = 16
F32 = mybir.dt.float32

# test.py can flip these; the grading harness leaves defaults.
TRACE = False
LAST_RESULT = None

_NC = None


def _build_nc():
    nc = bass.Bass()
    x = nc.dram_tensor("x", [R, D], F32, kind="ExternalInput")
    dec = nc.dram_tensor("dec", [R, D], F32, kind="ExternalInput")
    enc = nc.dram_tensor("enc", [R, E], F32, kind="ExternalInput")
    lat = nc.dram_tensor("lat", [R, I], F32, kind="ExternalInput")
    rsra = nc.dram_tensor("rsra", [E, I], F32, kind="ExternalInput")
    out = nc.dram_tensor("out", [P, S_COLS], F32, kind="ExternalOutput")

    Square = mybir.ActivationFunctionType.Square
    mult = mybir.AluOpType.mult
    bypass = mybir.AluOpType.bypass

    H = RT // 2  # half of the row tiles
    # enc halves, each [R/2, E] -> [128, H, E]
    enc_a = enc[0 : R // 2, :].rearrange("(n p) e -> p n e", p=P)
    enc_b = enc[R // 2 : R, :].rearrange("(n p) e -> p n e", p=P)
    lat_r = lat[:, :].rearrange("(n p) i -> p n i", p=P)  # [128, RT, I]

    ctx = contextlib.ExitStack()
    with ctx:
        xb = [
            ctx.enter_context(nc.sbuf_tensor(f"xb{t}", [P, D], F32)) for t in range(RT)
        ]
        db = [
            ctx.enter_context(nc.sbuf_tensor(f"db{t}", [P, D], F32)) for t in range(RT)
        ]
        enc_sb = ctx.enter_context(nc.sbuf_tensor([P, RT * E], F32))
        lat_sb = ctx.enter_context(nc.sbuf_tensor([P, RT * I], F32))
        rsra_sb = ctx.enter_context(nc.sbuf_tensor([E, I], F32))
        S = ctx.enter_context(nc.sbuf_tensor([P, S_COLS], F32))
        G_sb = ctx.enter_context(nc.sbuf_tensor([I, I], F32))
        scr_m = ctx.enter_context(nc.sbuf_tensor([E, I], F32))
        scr_i = ctx.enter_context(nc.sbuf_tensor([I, I], F32))
        scr_a = ctx.enter_context(nc.sbuf_tensor([E, I], F32))
        scr_e = ctx.enter_context(nc.sbuf_tensor([P, RT * E], F32))

        psum_M = ctx.enter_context(nc.psum_tensor([E, I], F32))
        psum_L = ctx.enter_context(nc.psum_tensor([I, I], F32))
        psum_G = ctx.enter_context(nc.psum_tensor([I, I], F32))

        s_x = [ctx.enter_context(nc.semaphore(f"s_x{t}")) for t in range(RT)]
        s_small = ctx.enter_context(nc.semaphore("s_small"))
        s_init = ctx.enter_context(nc.semaphore("s_init"))
        s_sub = ctx.enter_context(nc.semaphore("s_sub"))
        s_sq = ctx.enter_context(nc.semaphore("s_sq"))
        s_pe = ctx.enter_context(nc.semaphore("s_pe"))
        s_vfin = ctx.enter_context(nc.semaphore("s_vfin"))
        s_out = ctx.enter_context(nc.semaphore("s_out"))

        block = ctx.enter_context(nc.Block())

        @block.sync
        def _(sync):
            # SP HWDGE queue: first half of enc, then the x tiles
            sync.dma_start(
                out=enc_sb[:, 0 : H * E].rearrange("p (n e) -> p n e", n=H),
                in_=enc_a,
            ).then_inc(s_small, 16)
            for t in range(RT):
                sync.dma_start(
                    out=xb[t][:, :], in_=x[t * P : (t + 1) * P, :]
                ).then_inc(s_x[t], 16)
            # ship the accumulator once every column is final
            sync.wait_ge(s_sq, RT + 2)
            sync.wait_ge(s_vfin, 1)
            sync.dma_start(out=out[:, :], in_=S[:, :]).then_inc(s_out, 16)
            sync.wait_ge(s_out, 16)

        @block.scalar
        def _(scalar):
            # ACT HWDGE queue: small tensors, second enc half, then dec tiles
            scalar.dma_start(out=lat_sb[:, :].rearrange("p (n i) -> p n i", n=RT),
                             in_=lat_r).then_inc(s_small, 16)
            scalar.dma_start(out=rsra_sb[:, :], in_=rsra[:, :]).then_inc(s_small, 16)
            scalar.dma_start(
                out=enc_sb[:, H * E : RT * E].rearrange("p (n e) -> p n e", n=H),
                in_=enc_b,
            ).then_inc(s_small, 16)
            for t in range(RT):
                scalar.dma_start(
                    out=db[t][:, :], in_=dec[t * P : (t + 1) * P, :]
                ).then_inc(s_x[t], 16)
            # squares of the streamed differences
            scalar.wait_ge(s_init, 1)
            for t in range(RT):
                scalar.wait_ge(s_sub, t + 1)
                nc.scalar.activation(
                    out=db[t][:, :], in_=xb[t][:, :], func=Square,
                    accum_out=S[:, t : t + 1],
                ).then_inc(s_sq, 1)
                if t == 1:
                    # fill the idle gap with the small squares
                    scalar.wait_ge(s_small, 64)
                    nc.scalar.activation(
                        out=scr_e[:, :], in_=enc_sb[:, :], func=Square,
                        accum_out=S[:, 8:9],
                    ).then_inc(s_sq, 1)
                    nc.scalar.activation(
                        out=scr_a[:, :], in_=rsra_sb[:, :], func=Square,
                        accum_out=S[:E, 12:13],
                    ).then_inc(s_sq, 1)

        @block.vector
        def _(vector):
            nc.vector.memset(S[:, :], 0.0).then_inc(s_init, 1)
            # the big stream: d = x - dec, in place
            for t in range(RT):
                vector.wait_ge(s_x[t], 32)
                nc.vector.tensor_sub(xb[t][:, :], xb[t][:, :], db[t][:, :]).then_inc(
                    s_sub, 1
                )
            # tiny fused reductions over the PCA/proj matmul results
            vector.wait_ge(s_pe, 1)
            nc.vector.tensor_copy(G_sb[:, :], psum_G[:, :])
            nc.vector.scalar_tensor_tensor(
                out=scr_m[:, :], in0=psum_M[:, :], scalar=1.0, in1=rsra_sb[:, :],
                op0=bypass, op1=mult, accum_out=S[:E, 9:10],
            )
            nc.vector.scalar_tensor_tensor(
                out=scr_i[:, :], in0=psum_L[:, :], scalar=1.0, in1=G_sb[:, :],
                op0=bypass, op1=mult, accum_out=S[:I, 10:11],
            )
            nc.vector.scalar_tensor_tensor(
                out=scr_i[:, :], in0=G_sb[:, :], scalar=1.0, in1=G_sb[:, :],
                op0=bypass, op1=mult, accum_out=S[:I, 11:12],
            ).then_inc(s_vfin, 1)

        @block.tensor
        def _(tensor):
            tensor.wait_ge(s_small, 64)
            for t in range(RT):
                nc.tensor.matmul(
                    psum_M[:, :],
                    lhsT=enc_sb[:, t * E : (t + 1) * E],
                    rhs=lat_sb[:, t * I : (t + 1) * I],
                    start=(t == 0),
                    stop=(t == RT - 1),
                )
            for t in range(RT):
                nc.tensor.matmul(
                    psum_L[:, :],
                    lhsT=lat_sb[:, t * I : (t + 1) * I],
                    rhs=lat_sb[:, t * I : (t + 1) * I],
                    start=(t == 0),
                    stop=(t == RT - 1),
                )
            nc.tensor.matmul(
                psum_G[:, :], lhsT=rsra_sb[:, :], rhs=rsra_sb[:, :],
                start=True, stop=True,
            ).then_inc(s_pe, 1)

    return nc


def kernel(x, encoded, latent, decoded, rsrA):
    global _NC, LAST_RESULT
    if _NC is None:
        _NC = _build_nc()

    x = np.ascontiguousarray(x, dtype=np.float32)
    decoded = np.ascontiguousarray(decoded, dtype=np.float32)
    encoded = np.ascontiguousarray(encoded, dtype=np.float32)
    latent = np.ascontiguousarray(latent, dtype=np.float32)
    rsrA = np.ascontiguousarray(rsrA, dtype=np.float32)

    in_maps = []
    for c in range(N_CORES):
        sl = slice(c * R, (c + 1) * R)
        in_maps.append(
            {
                "x": x[sl],
                "dec": decoded[sl],
                "enc": encoded[sl],
                "lat": latent[sl],
                "rsra": rsrA,
            }
        )

    res = run_bass_kernel_spmd(_NC, in_maps, core_ids=list(range(N_CORES)), trace=TRACE)
    LAST_RESULT = res

    o = np.stack([r["out"] for r in res.results]).astype(np.float64)  # [8,128,16]
    cols = o.sum(axis=(0, 1))  # [16]
    s_recon = cols[0:8].sum()
    s_enc2 = cols[8]
    s_cross = cols[9]
    s_zsq = cols[10]
    g2 = o[0, :, 11].sum()
    ra2 = o[0, :, 12].sum()

    pca_sq = s_enc2 - 2.0 * s_cross + s_zsq
    proj_sq = g2 - 2.0 * ra2 + float(I)
    loss = s_recon / B + 1.1 * pca_sq / B + 0.1 * proj_sq / (I * I)
    return np.asarray(loss, dtype=np.float32)


# revision 7
# speedup vs baseline: 1.1178x; 1.0255x over previous
"""Trainium2 Bass kernel for the LELoss problem (raw Bass, 8-core SPMD).

loss = mean_b ||x_b - dec_b||^2
     + 1.1 * mean_b ||enc_b - (lat @ rsrA.T)_b||^2
     + 0.1 * mean((rsrA.T @ rsrA - I)^2)

(The knn/cdist/topk in the original module is dead code - its result is never
used - so the returned loss reduces to the three terms above.)

Per-core algebra (batch shard of R=1024 rows):
  sum||enc - lat@A.T||^2 = sum(enc^2) - 2*sum(M .* A) + sum(L .* G0)
      with M = enc.T @ lat [E,I], L = lat.T @ lat [I,I], G0 = A.T @ A [I,I]
  sum((G0 - I)^2) = sum(G0^2) - 2*sum(A^2) + I_dim
All partial sums land in columns of a [128,16] SBUF accumulator S which is
DMA'd out per core; the host collapses partitions/cores and applies weights.

DMA strategy: the two HWDGE queues (SP and ACT engines) each stream half of
the ~8.5MB/core so the ~415 GB/s/core HBM path is the only limiter. enc and
lat are loaded with a contiguous-rows layout ("(p n) d -> p n d": partition p
holds consecutive rows) so their descriptors are 2KB/640B instead of
512B/80B - M and L are sums over all rows, so any partition<->row assignment
works as long as enc and lat share it. The last x/dec tile is split into two
column halves so the final subtract/square pipeline over it, and the very
last square runs on DVE so the tail has no cross-engine hop.
"""

import contextlib

import numpy as np

try:
    import concourse.bass as bass
except ImportError:  # pragma: no cover - grading env fallback
    import sys

    sys.path.insert(0, "/opt/trn_rl_repo")
    import concourse.bass as bass

from concourse import mybir
from concourse.bass_utils import run_bass_kernel_spmd

N_CORES = 8
B, D, E, I = 8192, 1024, 128, 20
R = B // N_CORES  # rows per core = 1024
P = 128  # SBUF partitions
RT = R // P  # row tiles per core = 8
S_COLS = 16
F32 = mybir.dt.float32

TRACE = False
LAST_RESULT = None

_NC = None


def _build_nc():
    nc = bass.Bass()
    x = nc.dram_tensor("x", [R, D], F32, kind="ExternalInput")
    dec = nc.dram_tensor("dec", [R, D], F32, kind="ExternalInput")
    enc = nc.dram_tensor("enc", [R, E], F32, kind="ExternalInput")
    lat = nc.dram_tensor("lat", [R, I], F32, kind="ExternalInput")
    rsra = nc.dram_tensor("rsra", [E, I], F32, kind="ExternalInput")
    out = nc.dram_tensor("out", [P, S_COLS], F32, kind="ExternalOutput")

    Square = mybir.ActivationFunctionType.Square
    mult = mybir.AluOpType.mult
    bypass = mybir.AluOpType.bypass

    H = RT // 2
    # contiguous-rows layouts: partition p holds consecutive dram rows, so the
    # per-partition dram chunk is contiguous (2KB for enc halves, 640B for lat
    # halves). enc and lat use the same row assignment so matmul contractions
    # still pair row b of enc with row b of lat.
    enc_a = enc[0 : R // 2, :].rearrange("(p n) e -> p n e", p=P)  # rows 4p+n
    enc_b = enc[R // 2 : R, :].rearrange("(p n) e -> p n e", p=P)
    lat_a = lat[0 : R // 2, :].rearrange("(p n) i -> p n i", p=P)
    lat_b = lat[R // 2 : R, :].rearrange("(p n) i -> p n i", p=P)

    ctx = contextlib.ExitStack()
    with ctx:
        xb = [
            ctx.enter_context(nc.sbuf_tensor(f"xb{t}", [P, D], F32)) for t in range(RT)
        ]
        db = [
            ctx.enter_context(nc.sbuf_tensor(f"db{t}", [P, D], F32)) for t in range(RT)
        ]
        enc_sb = ctx.enter_context(nc.sbuf_tensor([P, RT * E], F32))
        lat_sb = ctx.enter_context(nc.sbuf_tensor([P, RT * I], F32))
        rsra_sb = ctx.enter_context(nc.sbuf_tensor([E, I], F32))
        S = ctx.enter_context(nc.sbuf_tensor([P, S_COLS], F32))
        G_sb = ctx.enter_context(nc.sbuf_tensor([I, I], F32))
        scr_m = ctx.enter_context(nc.sbuf_tensor([E, I], F32))
        scr_i = ctx.enter_context(nc.sbuf_tensor([I, I], F32))
        scr_a = ctx.enter_context(nc.sbuf_tensor([E, I], F32))
        scr_e = ctx.enter_context(nc.sbuf_tensor([P, RT * E], F32))

        psum_M = ctx.enter_context(nc.psum_tensor([E, I], F32))
        psum_L = ctx.enter_context(nc.psum_tensor([I, I], F32))
        psum_G = ctx.enter_context(nc.psum_tensor([I, I], F32))

        # pair sems: 0..6 whole tiles, 7 = tile7 first half, 8 = second half
        s_x = [ctx.enter_context(nc.semaphore(f"s_x{t}")) for t in range(RT + 1)]
        s_small = ctx.enter_context(nc.semaphore("s_small"))
        s_init = ctx.enter_context(nc.semaphore("s_init"))
        s_sub = ctx.enter_context(nc.semaphore("s_sub"))
        s_sq = ctx.enter_context(nc.semaphore("s_sq"))
        s_pe = ctx.enter_context(nc.semaphore("s_pe"))
        s_vfin = ctx.enter_context(nc.semaphore("s_vfin"))
        s_out = ctx.enter_context(nc.semaphore("s_out"))

        block = ctx.enter_context(nc.Block())

        DH = D // 2

        @block.sync
        def _(sync):
            # SP HWDGE queue (~4.34MB): lat + rsra + first enc half + x tiles
            sync.dma_start(
                out=lat_sb[:, 0 : H * I].rearrange("p (n i) -> p n i", n=H),
                in_=lat_a,
            ).then_inc(s_small, 16)
            sync.dma_start(
                out=lat_sb[:, H * I : RT * I].rearrange("p (n i) -> p n i", n=H),
                in_=lat_b,
            ).then_inc(s_small, 16)
            sync.dma_start(out=rsra_sb[:, :], in_=rsra[:, :]).then_inc(s_small, 16)
            sync.dma_start(
                out=enc_sb[:, 0 : H * E].rearrange("p (n e) -> p n e", n=H),
                in_=enc_a,
            ).then_inc(s_small, 16)
            for t in range(RT - 1):
                sync.dma_start(
                    out=xb[t][:, :], in_=x[t * P : (t + 1) * P, :]
                ).then_inc(s_x[t], 16)
            t = RT - 1
            sync.dma_start(
                out=xb[t][:, 0:DH], in_=x[t * P : (t + 1) * P, 0:DH]
            ).then_inc(s_x[7], 16)
            sync.dma_start(
                out=xb[t][:, DH:D], in_=x[t * P : (t + 1) * P, DH:D]
            ).then_inc(s_x[8], 16)
            # ship the accumulator once every column is final
            sync.wait_ge(s_sq, 10)
            sync.wait_ge(s_vfin, 2)
            sync.dma_start(out=out[:, :], in_=S[:, :]).then_inc(s_out, 16)
            sync.wait_ge(s_out, 16)

        @block.scalar
        def _(scalar):
            # ACT HWDGE queue (~4.25MB): second enc half + dec tiles
            scalar.dma_start(
                out=enc_sb[:, H * E : RT * E].rearrange("p (n e) -> p n e", n=H),
                in_=enc_b,
            ).then_inc(s_small, 16)
            for t in range(RT - 1):
                scalar.dma_start(
                    out=db[t][:, :], in_=dec[t * P : (t + 1) * P, :]
                ).then_inc(s_x[t], 16)
            t = RT - 1
            scalar.dma_start(
                out=db[t][:, 0:DH], in_=dec[t * P : (t + 1) * P, 0:DH]
            ).then_inc(s_x[7], 16)
            scalar.dma_start(
                out=db[t][:, DH:D], in_=dec[t * P : (t + 1) * P, DH:D]
            ).then_inc(s_x[8], 16)
            # squares of the streamed differences (tiles 0..6 and 7a)
            scalar.wait_ge(s_init, 1)
            for t in range(RT):
                scalar.wait_ge(s_sub, t + 1)
                if t < RT - 1:
                    nc.scalar.activation(
                        out=db[t][:, :], in_=xb[t][:, :], func=Square,
                        accum_out=S[:, t : t + 1],
                    ).then_inc(s_sq, 1)
                else:
                    nc.scalar.activation(
                        out=db[t][:, 0:DH], in_=xb[t][:, 0:DH], func=Square,
                        accum_out=S[:, t : t + 1],
                    ).then_inc(s_sq, 1)
                if t == 1:
                    # fill the idle gap with the small squares
                    scalar.wait_ge(s_small, 80)
                    nc.scalar.activation(
                        out=scr_e[:, :], in_=enc_sb[:, :], func=Square,
                        accum_out=S[:, 8:9],
                    ).then_inc(s_sq, 1)
                    nc.scalar.activation(
                        out=scr_a[:, :], in_=rsra_sb[:, :], func=Square,
                        accum_out=S[:E, 12:13],
                    ).then_inc(s_sq, 1)

        @block.vector
        def _(vector):
            nc.vector.memset(S[:, :], 0.0).then_inc(s_init, 1)
            # the big stream: d = x - dec, in place
            for t in range(RT - 1):
                vector.wait_ge(s_x[t], 32)
                nc.vector.tensor_sub(xb[t][:, :], xb[t][:, :], db[t][:, :]).then_inc(
                    s_sub, 1
                )
            # tiny fused reductions over the PCA/proj matmul results, slotted
            # into the gap before the last tile's halves arrive
            vector.wait_ge(s_pe, 1)
            nc.vector.tensor_copy(G_sb[:, :], psum_G[:, :])
            nc.vector.scalar_tensor_tensor(
                out=scr_m[:, :], in0=psum_M[:, :], scalar=1.0, in1=rsra_sb[:, :],
                op0=bypass, op1=mult, accum_out=S[:E, 9:10],
            )
            nc.vector.scalar_tensor_tensor(
                out=scr_i[:, :], in0=psum_L[:, :], scalar=1.0, in1=G_sb[:, :],
                op0=bypass, op1=mult, accum_out=S[:I, 10:11],
            )
            nc.vector.scalar_tensor_tensor(
                out=scr_i[:, :], in0=G_sb[:, :], scalar=1.0, in1=G_sb[:, :],
                op0=bypass, op1=mult, accum_out=S[:I, 11:12],
            ).then_inc(s_vfin, 1)
            # last tile, two column halves; the second half's square also runs
            # here so the tail never leaves DVE
            t = RT - 1
            vector.wait_ge(s_x[7], 32)
            nc.vector.tensor_sub(
                xb[t][:, 0:DH], xb[t][:, 0:DH], db[t][:, 0:DH]
            ).then_inc(s_sub, 1)
            vector.wait_ge(s_x[8], 32)
            nc.vector.tensor_sub(xb[t][:, DH:D], xb[t][:, DH:D], db[t][:, DH:D])
            nc.vector.scalar_tensor_tensor(
                out=db[t][:, DH:D], in0=xb[t][:, DH:D], scalar=1.0,
                in1=xb[t][:, DH:D], op0=bypass, op1=mult,
                accum_out=S[:, 13:14],
            ).then_inc(s_vfin, 1)

        @block.tensor
        def _(tensor):
            tensor.wait_ge(s_small, 80)
            for t in range(RT):
                nc.tensor.matmul(
                    psum_M[:, :],
                    lhsT=enc_sb[:, t * E : (t + 1) * E],
                    rhs=lat_sb[:, t * I : (t + 1) * I],
                    start=(t == 0),
                    stop=(t == RT - 1),
                )
            for t in range(RT):
                nc.tensor.matmul(
                    psum_L[:, :],
                    lhsT=lat_sb[:, t * I : (t + 1) * I],
                    rhs=lat_sb[:, t * I : (t + 1) * I],
                    start=(t == 0),
                    stop=(t == RT - 1),
                )
            nc.tensor.matmul(
                psum_G[:, :], lhsT=rsra_sb[:, :], rhs=rsra_sb[:, :],
                start=True, stop=True,
            ).then_inc(s_pe, 1)

    return nc


def kernel(x, encoded, latent, decoded, rsrA):
    global _NC, LAST_RESULT
    if _NC is None:
        _NC = _build_nc()

    x = np.ascontiguousarray(x, dtype=np.float32)
    decoded = np.ascontiguousarray(decoded, dtype=np.float32)
    encoded = np.ascontiguousarray(encoded, dtype=np.float32)
    latent = np.ascontiguousarray(latent, dtype=np.float32)
    rsrA = np.ascontiguousarray(rsrA, dtype=np.float32)

    in_maps = []
    for c in range(N_CORES):
        sl = slice(c * R, (c + 1) * R)
        in_maps.append(
            {
                "x": x[sl],
                "dec": decoded[sl],
                "enc": encoded[sl],
                "lat": latent[sl],
                "rsra": rsrA,
            }
        )

    res = run_bass_kernel_spmd(_NC, in_maps, core_ids=list(range(N_CORES)), trace=TRACE)
    LAST_RESULT = res

    o = np.stack([r["out"] for r in res.results]).astype(np.float64)  # [8,128,16]
    cols = o.sum(axis=(0, 1))  # [16]
    s_recon = cols[0:8].sum() + cols[13]
    s_enc2 = cols[8]
    s_cross = cols[9]
    s_zsq = cols[10]
    g2 = o[0, :, 11].sum()
    ra2 = o[0, :, 12].sum()

    pca_sq = s_enc2 - 2.0 * s_cross + s_zsq
    proj_sq = g2 - 2.0 * ra2 + float(I)
    loss = s_recon / B + 1.1 * pca_sq / B + 0.1 * proj_sq / (I * I)
    return np.asarray(loss, dtype=np.float32)
